# revision 1
# baseline (speedup 1.0000x reference)
"""Trainium2 Bass kernel for the CGIM sparse-attention block.

Per-sample math (reference):
  Qf = Wq @ [F1;F2] + bq            (1x1 conv, transposed-layout on device)
  Qs = softmax_d(Qf per head)
  per branch i: K = Wk_i @ F_i (+bk_i cancels), V = Wv_i @ F_i + bv_i
                Ks = softmax_hw(K);  Att = Ks @ Qs;  Xw = Att @ V
  fused = concat(mu*X1 + F1, mu*X2 + F2)
  y = relu(BN(conv3x3(fused, Wc)))

Sharding: data-parallel over batch (B=8) across the 8 NeuronCores; weights
replicated. Each core computes one sample end to end; no collectives.

v2 device-side algebra (on top of the v1 tricks):
 - All attention-side matmuls (Q/K/V/ssum/Att) run in fp8 e4m3 DoubleRow
   mode (K=256 per matmul, 2x PE throughput). Weights are host-scaled by
   64 into fp8 normal range; the 1/64 is folded into the exp/identity
   ACT scale. Normalized Qs is stored x16 (fp8 range), compensated in
   muv = mu/16. exp(K) <= ~122 < 240 so the ACT writes it to fp8
   directly; exp(Q) must be normalized in bf16 first (can exceed 240).
 - conv3x3 is computed as 1-D row Winograd F(2,3): per 2 output rows,
   4 multiply-rows M_k over (4 ci x 3 dx) shifted windows = 48 matmuls
   per 16 image rows instead of 72. Row-transform R_k on DVE/GpSimd,
   A^T-combine (2 adds per output row) on DVE, BN+ReLU in the ACT.
 - SBUF: fbf/qk8/vsb live in a pool that closes after Xw so the 68KB
   Winograd R tile can reuse their space.
All matmul accumulation in fp32 PSUM.
"""

import numpy as np
import ml_dtypes

import concourse.bass as bass
import concourse.mybir as mybir
import concourse.tile as tile
from concourse import bacc
from concourse.bass_utils import run_bass_kernel_spmd

BF16 = mybir.dt.bfloat16
F8 = mybir.dt.float8e4
F32 = mybir.dt.float32
AF = mybir.ActivationFunctionType
ALU = mybir.AluOpType
AX = mybir.AxisListType
DR = mybir.MatmulPerfMode.DoubleRow

B, C, H, W = 8, 256, 64, 64
HW = H * W                  # 4096
NH, D = 8, 32               # heads, per-head dim
NT = HW // 128              # 32 hw-tiles of 128
PH, PW = H + 2, 72          # padded conv image rows x alloc cols
RW = 68                     # R tile col alloc (66 used)
N_CORES = 8
BN_EPS = 1e-5
QS_SCALE = 16.0             # Qs stored x16 in fp8; muv = mu/16
WSC = 64.0                  # fp8 weight scale; 1/64 folded into ACT scale

_bf = ml_dtypes.bfloat16
_e4 = ml_dtypes.float8_e4m3


def _build_program() -> bass.Bass:
    nc = bacc.Bacc("TRN2", target_bir_lowering=False)

    # ---- DRAM I/O (per core) ----
    fb_d = nc.dram_tensor("fb", [128, 4, HW], BF16, kind="ExternalInput").ap()
    f8_d = nc.dram_tensor("f8", [128, 4, HW], F8, kind="ExternalInput").ap()
    wqk1_d = nc.dram_tensor("wqk1", [128, 2, 512], F8,
                            kind="ExternalInput").ap()
    wqk2_d = nc.dram_tensor("wqk2", [128, 2, 512], F8,
                            kind="ExternalInput").ap()
    wv_d = nc.dram_tensor("wv", [128, 2, 2, 256], F8, kind="ExternalInput").ap()
    gw_d = nc.dram_tensor("gw", [128, 4, 24, 128], BF16,
                          kind="ExternalInput").ap()
    bq_d = nc.dram_tensor("bq", [1, 256], BF16, kind="ExternalInput").ap()
    bv_d = nc.dram_tensor("bv", [128, 2, 2], F32, kind="ExternalInput").ap()
    bns_d = nc.dram_tensor("bns", [128, 2], F32, kind="ExternalInput").ap()
    bnb_d = nc.dram_tensor("bnb", [128, 2], F32, kind="ExternalInput").ap()
    muv_d = nc.dram_tensor("muv", [128, 1], F32, kind="ExternalInput").ap()
    y_d = nc.dram_tensor("y", [C, H, W], F32, kind="ExternalOutput").ap()

    with tile.TileContext(nc) as tc:
        with tc.tile_pool(name="per", bufs=1) as per, \
             tc.tile_pool(name="sml", bufs=4) as sml:

            # ---- persistent SBUF tiles ----
            wqk1 = per.tile([128, 2, 512], F8)
            wqk2 = per.tile([128, 2, 512], F8)
            wv = per.tile([128, 2, 2, 256], F8)
            gw = per.tile([128, 4, 24, 128], BF16)
            bq = per.tile([1, 256], BF16)
            bv = per.tile([128, 2, 2], F32)
            bns = per.tile([128, 2], F32)
            bnb = per.tile([128, 2], F32)
            muv = per.tile([128, 1], F32)

            ones_row = per.tile([1, 128], BF16)
            nc.vector.memset(ones_row, 1.0)
            # DR ldweights needs a 16B-aligned k-subtile stride
            ones2t = per.tile([128, 2, 16], F8)
            nc.gpsimd.memset(ones2t, 1.0)
            ones2 = ones2t[:, :, 0:1]

            f8 = per.tile([128, 4, HW], F8)           # matmul operand F
            fp = [per.tile([128, PH, PW], BF16, tag=f"fp{j}", name=f"fp{j}")
                  for j in range(4)]

            vp_ctx = tc.tile_pool(name="vp", bufs=1)
            vp = vp_ctx.__enter__()
            fbf = vp.tile([128, 4, HW], BF16)         # epilogue-add F (bf16)
            # qk8[:, n, 0:256]=16*Qs, [256:512]=exp(K1), [512:768]=exp(K2)
            qk8 = vp.tile([128, NT, 768], F8)
            vsb1 = vp.tile([128, 2, HW], BF16)        # V1, 2 m-groups
            vsb2 = vp.tile([128, 2, HW], BF16)

            # ---- preamble DMAs: f8 + wqk are the critical path
            nc.scalar.dma_start(wqk1, wqk1_d)
            nc.scalar.dma_start(wqk2, wqk2_d)
            nc.scalar.dma_start(bq, bq_d)
            for ch in range(4):
                eng = nc.sync if ch < 2 else nc.scalar
                eng.dma_start(f8[:, :, ch * 1024:(ch + 1) * 1024],
                              f8_d[:, :, ch * 1024:(ch + 1) * 1024])
            nc.gpsimd.dma_start(wv, wv_d)
            nc.gpsimd.dma_start(bv, bv_d)
            nc.gpsimd.dma_start(muv, muv_d)
            # fbf not needed until the Xw epilogue; sync queue, after f8
            for ch in range(4):
                nc.sync.dma_start(fbf[:, :, ch * 1024:(ch + 1) * 1024],
                                  fb_d[:, :, ch * 1024:(ch + 1) * 1024])
            # fp pad-only memsets (rows 0,65 and cols 0,65)
            for j in range(4):
                nc.vector.memset(fp[j][:, 0:PH:PH - 1, 0:66], 0.0)
                nc.vector.memset(fp[j][:, 1:65, 0:66:65], 0.0)

            # ================= Phase 1: Q/K exps + softmax pieces (fp8 DR)
            # pv opens first so the scheduler can interleave V matmuls into
            # phase-1 PE gaps (V depends only on f8 + wv, ready early).
            pv_ctx = tc.tile_pool(name="pv", bufs=3, space="PSUM")
            pv = pv_ctx.__enter__()
            with tc.tile_pool(name="pq", bufs=2, space="PSUM") as pq, \
                 tc.tile_pool(name="pss", bufs=1, space="PSUM") as pss:

                ps_s = pss.tile([1, 512], F32, tag="s")

                LAG = 2

                def emit_ssum(p):
                    nc.tensor.matmul(ps_s, ones2,
                                     qk8[:, 2 * p:2 * p + 2, 256:768],
                                     start=(p == 0), stop=(p == NT // 2 - 1),
                                     perf_mode=DR)

                gk = dict(skip_group_check=True)
                for n in range(NT):
                    # PSUM layout [Q | K1 | K2]: one 512-col [Q|K1] DR
                    # matmul fills bank A (pair0 lhsT); the pair1 lhsT then
                    # feeds a 256-col Q-accumulate and the K2 matmul (bank B,
                    # its own start). Matmul outs never cross a PSUM bank.
                    pqk = pq.tile([128, 768], F32, tag="qk")
                    psq = pqk[:, 0:256]
                    nsl = slice(n * 128, (n + 1) * 128)
                    nc.tensor.matmul(pqk[:, 0:512], f8[:, 0:2, nsl], wqk1,
                                     start=True, stop=False, perf_mode=DR, **gk)
                    nc.tensor.matmul(psq, f8[:, 2:4, nsl], wqk2[:, :, 0:256],
                                     start=False, stop=False, perf_mode=DR, **gk)
                    nc.tensor.matmul(pqk[:, 512:768], f8[:, 2:4, nsl],
                                     wqk2[:, :, 256:512],
                                     start=True, stop=False, perf_mode=DR, **gk)
                    nc.tensor.matmul(psq, ones_row, bq, start=False, stop=True,
                                     **gk)

                    # exp(K)/64 straight to fp8 (max ~122 < 240), writing the
                    # (g-major, br-minor) interleaved qk8 column layout
                    kin = pqk[:, 256:768].rearrange(
                        "p (b g c) -> p b g c", b=2, g=2)
                    kout = qk8[:, n, 256:768].rearrange(
                        "p (g b c) -> p b g c", g=2, b=2)
                    actk = nc.scalar.activation(kout, kin, AF.Exp,
                                                scale=1.0 / WSC)
                    # exp(Q) to bf16, normalize per head, write fp8 x16
                    qbf = sml.tile([128, 256], BF16, tag="qbf")
                    actq = nc.scalar.activation(qbf, psq, AF.Exp,
                                                scale=1.0 / WSC)
                    if n == 8:
                        gw_anchor = actq
                    q3 = qbf.rearrange("p (h e) -> p h e", h=NH)
                    rq = sml.tile([128, NH], F32, tag="rq")
                    nc.vector.tensor_reduce(rq, q3, axis=AX.X, op=ALU.add)
                    rr = sml.tile([128, NH], F32, tag="rr")
                    nc.vector.reciprocal(rr, rq)
                    q83 = qk8[:, n, 0:256].rearrange("p (h e) -> p h e", h=NH)
                    if n % 2 == 0:
                        nc.vector.scalar_tensor_tensor(
                            out=q83, in0=q3, scalar=QS_SCALE,
                            in1=rr.to_broadcast([128, NH, D]),
                            op0=ALU.mult, op1=ALU.mult)
                    else:
                        rr16 = sml.tile([128, NH], F32, tag="rr16")
                        nc.vector.tensor_scalar_mul(rr16, rr, QS_SCALE)
                        nc.gpsimd.tensor_tensor(
                            q83, q3, rr16.to_broadcast([128, NH, D]),
                            ALU.mult)

                    if n >= 2 * LAG + 1 and n % 2 == 1:
                        emit_ssum((n - 1) // 2 - LAG)
                for p in range(NT // 2 - LAG, NT // 2):
                    emit_ssum(p)

                # 1/S row -> per-partition columns (tiny SBUF->SBUF DMAs)
                scale = {}
                rs = sml.tile([1, 512], F32, tag="rs")
                nc.vector.reciprocal(rs, ps_s)
                for br in range(2):
                    for m in range(2):
                        col = sml.tile([128, 1], F32, tag="scat")
                        nc.sync.dma_start(
                            col, rs[0:1, m * 256 + br * 128:
                                    m * 256 + (br + 1) * 128])
                        sc = sml.tile([128, 1], F32, tag="scale")
                        nc.vector.tensor_mul(sc, col, muv)   # mu/(16*S_d)
                        scale[(br, m)] = sc

            # Winograd weights: start loading mid-phase-1 (nosync dep keeps
            # them out of the preamble DMA window), done before the conv.
            from concourse.tile import add_dep_helper
            for h in range(2):
                d = nc.sync.dma_start(gw[:, :, h * 12:(h + 1) * 12, :],
                                      gw_d[:, :, h * 12:(h + 1) * 12, :])
                add_dep_helper(d.ins, gw_anchor.ins, sync=False,
                               reason="defer gw load past preamble")
            d = nc.sync.dma_start(bns, bns_d)
            add_dep_helper(d.ins, gw_anchor.ins, sync=False, reason="defer")
            d = nc.sync.dma_start(bnb, bnb_d)
            add_dep_helper(d.ins, gw_anchor.ins, sync=False, reason="defer")

            # ================= Phase 2: V (fp8 DR) + Att (fp8 DR) + blockdiag
            def emit_v(pv, br, vsb):
                for m in range(2):
                    for n8 in range(8):
                        psv = pv.tile([128, 512], F32, tag="v",
                                      name=f"psv{br}{m}{n8}")
                        nc.tensor.matmul(
                            psv, wv[:, br, :, m * 128:(m + 1) * 128],
                            f8[:, 2 * br:2 * br + 2,
                               n8 * 512:(n8 + 1) * 512],
                            start=True, stop=True, perf_mode=DR)
                        nc.scalar.activation(
                            vsb[:, m, n8 * 512:(n8 + 1) * 512], psv,
                            AF.Identity, bias=bv[:, br, m:m + 1],
                            scale=1.0 / WSC)

            emit_v(pv, 0, vsb1)

            with tc.tile_pool(name="pa", bufs=4, space="PSUM") as pa:
                psa = {}
                for g in range(2):
                    psa[g] = pa.tile([128, 256], F32, tag="a", name=f"psa{g}")
                for g in range(2):
                    for p in range(NT // 2):
                        nc.tensor.matmul(
                            psa[g],
                            qk8[:, 2 * p:2 * p + 2, g * 128:(g + 1) * 128],
                            qk8[:, 2 * p:2 * p + 2,
                                256 + g * 256:256 + (g + 1) * 256],
                            start=(p == 0), stop=(p == NT // 2 - 1),
                            perf_mode=DR)

                emit_v(pv, 1, vsb2)

                attbd = {}
                for g in range(2):
                    for br in range(2):
                        t = sml.tile([128, 128], BF16, tag="attbd")
                        nc.vector.memset(t, 0.0)
                        for hb in range(4):
                            hs = slice(hb * 32, (hb + 1) * 32)
                            nc.any.tensor_copy(
                                t[hs, hs], psa[g][hs, br * 128 + hb * 32:
                                                  br * 128 + (hb + 1) * 32])
                        attbd[(br, g)] = t
            pv_ctx.__exit__(None, None, None)

            # ================= Phase 2b: Xw + fused epilogue (bf16)
            with tc.tile_pool(name="px", bufs=6, space="PSUM") as px:
                for br, vsb in enumerate((vsb1, vsb2)):
                    for g in range(2):
                        for n8 in range(8):
                            pxt = px.tile([128, 512], F32, tag="x")
                            nc.tensor.matmul(
                                pxt, attbd[(br, g)],
                                vsb[:, g, n8 * 512:(n8 + 1) * 512],
                                start=True, stop=True)
                            # fused = (Xw_raw * mu/(16 S_d)) + F -> padded
                            j = 2 * br + g
                            out = fp[j][:, 1 + n8 * 8:9 + n8 * 8, 1:65]
                            nc.vector.scalar_tensor_tensor(
                                out=out,
                                in0=pxt, scalar=scale[(br, g)],
                                in1=fbf[:, j, n8 * 512:(n8 + 1) * 512],
                                op0=ALU.mult, op1=ALU.add)
            vp_ctx.__exit__(None, None, None)

            # ================= Phase 3: 1-D Winograd conv + BN + ReLU
            with tc.tile_pool(name="rp", bufs=1) as rp, \
                 tc.tile_pool(name="pc", bufs=8, space="PSUM") as pc, \
                 tc.tile_pool(name="ep", bufs=2) as ep:
                R = rp.tile([128, 4, 4, 32, RW], BF16)
                # R_k row combos: k0=d0-d2, k1=d1+d2, k2=d2-d1, k3=d1-d3.
                # Chunked per tyc so conv groups can start as R data lands,
                # and ci-major so early branches overlap the Xw phase.
                combos = [(0, 2, ALU.subtract), (1, 2, ALU.add),
                          (2, 1, ALU.subtract), (1, 3, ALU.subtract)]
                for ci in range(4):
                    for tyc in range(4):
                        for k, (a, b, op) in enumerate(combos):
                            r0 = a + 16 * tyc
                            r1 = b + 16 * tyc
                            nc.vector.tensor_tensor(
                                R[:, ci, k, tyc * 8:(tyc + 1) * 8, 0:66],
                                fp[ci][:, r0:min(r0 + 16, PH):2, 0:66],
                                fp[ci][:, r1:min(r1 + 16, PH):2, 0:66], op)

                groups = [(ty0, m, nty)
                          for ty0, nty in ((0, 8), (8, 8), (16, 8), (24, 8))
                          for m in range(2)]
                for ty0, m, nty in groups:
                    fw = nty * 64
                    pst_t = [pc.tile([128, 512], F32, tag="c",
                                     name=f"psc{m}{ty0}{k}")
                             for k in range(4)]
                    pst = [t[:, 0:fw] for t in pst_t]
                    for k in range(4):
                        for ci in range(4):
                            for dx in range(3):
                                nc.tensor.matmul(
                                    pst[k],
                                    gw[:, ci, k * 6 + dx * 2 + m, :],
                                    R[:, ci, k, ty0:ty0 + nty, dx:dx + 64],
                                    start=(ci == 0 and dx == 0),
                                    stop=(ci == 3 and dx == 2))
                    # A^T combine: y0 = M0+M1+M2, y1 = M1-M2-M3.
                    # DVE has one PSUM read port: stage M1 in SBUF so
                    # every TT reads at most one PSUM operand.
                    m1s_t = ep.tile([128, 512], F32, tag="m1s",
                                    name=f"m1s{m}{ty0}")
                    m1s = m1s_t[:, 0:fw]
                    nc.scalar.activation(m1s, pst[1], AF.Identity)
                    for i in range(2):
                        t_t = ep.tile([128, 512], F32, tag=f"t{i}",
                                      name=f"t{i}_{m}{ty0}")
                        t = t_t[:, 0:fw]
                        if i == 0:
                            nc.vector.tensor_tensor(t, pst[0], m1s, ALU.add)
                            s_t = ep.tile([128, 512], F32, tag="s0",
                                          name=f"s0_{m}{ty0}")
                            s = s_t[:, 0:fw]
                            nc.vector.tensor_tensor(s, t, pst[2], ALU.add)
                        else:
                            nc.vector.tensor_tensor(t, m1s, pst[2],
                                                    ALU.subtract)
                            s_t = ep.tile([128, 512], F32, tag="s1",
                                          name=f"s1_{m}{ty0}")
                            s = s_t[:, 0:fw]
                            nc.vector.tensor_tensor(s, t, pst[3],
                                                    ALU.subtract)
                        ysb_t = sml.tile([128, 512], F32, tag="y",
                                         name=f"y{m}{ty0}{i}")
                        ysb = ysb_t[:, 0:fw]
                        nc.scalar.activation(ysb, s, AF.Relu,
                                             bias=bnb[:, m:m + 1],
                                             scale=bns[:, m:m + 1])
                        eng = nc.sync if (ty0 + i) % 2 == 0 else nc.gpsimd
                        eng.dma_start(
                            y_d[m * 128:(m + 1) * 128,
                                ty0 * 2 + i:(ty0 + nty) * 2:2, :],
                            ysb.rearrange("p (a b) -> p a b", a=nty))
    nc.compile()
    return nc


_PROGRAM = None


def _get_program():
    global _PROGRAM
    if _PROGRAM is None:
        _PROGRAM = _build_program()
    return _PROGRAM


def kernel(F1, F2, Wq, bq, Wk1, bk1, Wv1, bv1, Wk2, bk2, Wv2, bv2,
           mu, Wc, gamma, beta, rmean, rvar):
    import os
    import sys
    if "antenv.axon_hooks" not in sys.modules:
        try:
            import antenv.axon_hooks  # noqa: F401
        except ImportError:
            # no profiling hook available: make sure a stray BASS_TRACE
            # can't route run_bass_kernel_spmd into the hook import
            os.environ["BASS_NEVER_TRACE"] = "1"
    f32 = np.float32
    F1 = np.asarray(F1, f32)
    F2 = np.asarray(F2, f32)

    def tile_T(w):   # [O, Cin] -> [128, Cin//128, O] f32 (lhsT tiles)
        wt = np.ascontiguousarray(np.asarray(w, f32).T)      # [Cin, O]
        cin, o = wt.shape
        return wt.reshape(cin // 128, 128, o).transpose(1, 0, 2)

    def q8(w):
        return np.ascontiguousarray(w * WSC).astype(_e4)

    wq_t = tile_T(Wq)                                        # [128, 4, 256]
    # fused phase-1 weights: rhs for [K1|Q] (F1-pair) and [Q|K2] (F2-pair)
    wqk1_h = q8(np.concatenate([wq_t[:, 0:2, :], tile_T(Wk1)], axis=2))
    wqk2_h = q8(np.concatenate([wq_t[:, 2:4, :], tile_T(Wk2)], axis=2))
    wv_h = q8(np.stack([tile_T(Wv1), tile_T(Wv2)], axis=1))

    Wc = np.asarray(Wc, f32)                                 # [256, 512, 3, 3]
    # 1-D Winograd dy-combos: G rows applied to the 3 dy taps
    g0 = Wc[:, :, 0, :]
    g1 = (Wc[:, :, 0, :] + Wc[:, :, 1, :] + Wc[:, :, 2, :]) * 0.5
    g2 = (Wc[:, :, 0, :] - Wc[:, :, 1, :] + Wc[:, :, 2, :]) * 0.5
    g3 = Wc[:, :, 2, :]
    G4 = np.stack([g0, g1, g2, g3])                          # [4k, 256, 512, 3]
    # gw[p, ci, k*6+dx*2+m, col] = G4[k, m*128+col, ci*128+p, dx]
    gw_h = G4.reshape(4, 2, 128, 4, 128, 3)                  # k,m,col,ci,p,dx
    gw_h = gw_h.transpose(4, 3, 0, 5, 1, 2)                  # p,ci,k,dx,m,col
    gw_h = np.ascontiguousarray(gw_h.reshape(128, 4, 24, 128)).astype(_bf)

    bq_h = (np.asarray(bq, f32) * WSC).reshape(1, 256).astype(_bf)
    # bv_h[p, br, m] = bv_br[m*128 + p]
    bv_h = np.ascontiguousarray(
        np.stack([np.asarray(bv1, f32), np.asarray(bv2, f32)],
                 axis=0).reshape(2, 2, 128).transpose(2, 0, 1))
    inv = np.asarray(gamma, f32) / np.sqrt(np.asarray(rvar, f32) + BN_EPS)
    b2 = np.asarray(beta, f32) - np.asarray(rmean, f32) * inv
    bns_h = np.ascontiguousarray(inv.reshape(2, 128).T)      # [128, 2]
    bnb_h = np.ascontiguousarray(b2.reshape(2, 128).T)
    muv_h = np.full((128, 1), np.asarray(mu, f32).reshape(-1)[0] / QS_SCALE,
                    f32)

    shared = dict(wqk1=wqk1_h, wqk2=wqk2_h, wv=wv_h, gw=gw_h, bq=bq_h,
                  bv=bv_h, bns=bns_h, bnb=bnb_h, muv=muv_h)

    def packF(b):
        f1r = F1[b].reshape(C, HW)
        f2r = F2[b].reshape(C, HW)
        st = np.stack([f1r[:128], f1r[128:], f2r[:128], f2r[128:]], axis=1)
        return np.ascontiguousarray(st)                      # [128, 4, HW]

    in_maps = []
    for b in range(N_CORES):
        fb = packF(b)
        in_maps.append(dict(fb=fb.astype(_bf), f8=fb.astype(_e4), **shared))

    nc = _get_program()
    res = run_bass_kernel_spmd(nc, in_maps, list(range(N_CORES)))
    kernel.last_results = res

    out = np.stack([res.results[b]["y"] for b in range(N_CORES)])
    return out.reshape(B, C, H, W)


kernel.last_results = None



# revision 6
# speedup vs baseline: 1.0100x; 1.0100x over previous
"""Trainium2 Bass kernel for the CGIM sparse-attention block.

Per-sample math (reference):
  Qf = Wq @ [F1;F2] + bq            (1x1 conv, transposed-layout on device)
  Qs = softmax_d(Qf per head)
  per branch i: K = Wk_i @ F_i (+bk_i cancels), V = Wv_i @ F_i + bv_i
                Ks = softmax_hw(K);  Att = Ks @ Qs;  Xw = Att @ V
  fused = concat(mu*X1 + F1, mu*X2 + F2)
  y = relu(BN(conv3x3(fused, Wc)))

Sharding: data-parallel over batch (B=8) across the 8 NeuronCores; weights
replicated. Each core computes one sample end to end; no collectives.

v3 device-side algebra (on top of v2):
 - Q-bias matmuls eliminated: exp(q+b) = exp(q)*exp(b).  The exp(b) factor
   moves into the V copy's per-partition scale/bias (Xw sums over the same
   e index), and the softmax denominator S_q becomes a weighted sum
   computed as qw = exp(q)*expb (GpSimd TT) -> tensor_reduce.
 - Att accumulation (fp8 DR) is pipelined INTO phase 1 with a LAG, like
   ssum, into a single [128,512] PSUM tile; only the block-diag extraction
   remains after the loop.
 - Xw runs in fp8 DoubleRow: att8 lhsT is [128,2,128] fp8 (zero subtile for
   the other g-group), vsb8 is the fp8 [128,2,HW] stacked V with exp(bq)
   and all static scales folded in.  pxt = 8*S_K*Xw_true, so muv = mu/8.
 - conv3x3 is 1-D row Winograd F(2,3) as in v2.
 - f8/fbf/qk8/vsb8/att8/expb live in a pool that closes after Xw so the
   Winograd R tile reuses their space.
All matmul accumulation in fp32 PSUM.
"""

import numpy as np
import ml_dtypes

import concourse.bass as bass
import concourse.mybir as mybir
import concourse.tile as tile
from concourse import bacc
from concourse.bass_utils import run_bass_kernel_spmd

BF16 = mybir.dt.bfloat16
F8 = mybir.dt.float8e4
F32 = mybir.dt.float32
AF = mybir.ActivationFunctionType
ALU = mybir.AluOpType
AX = mybir.AxisListType
DR = mybir.MatmulPerfMode.DoubleRow

B, C, H, W = 8, 256, 64, 64
HW = H * W                  # 4096
NH, D = 8, 32               # heads, per-head dim
NT = HW // 128              # 32 hw-tiles of 128
PH, PW = H + 2, 72          # padded conv image rows x alloc cols
RW = 68                     # R tile col alloc (66 used)
N_CORES = 8
BN_EPS = 1e-5
QS_SCALE = 16.0             # qs8 stored x16 in fp8
WSC = 64.0                  # fp8 weight scale; 1/64 folded into ACT scale
ATT_DS = 64.0               # att8 = psa/64
V_US = 32.0                 # vsb8 = 32*exp(bq)*V

_bf = ml_dtypes.bfloat16
_e4 = ml_dtypes.float8_e4m3


def _build_program() -> bass.Bass:
    nc = bacc.Bacc("TRN2", target_bir_lowering=False)

    # ---- DRAM I/O (per core) ----
    fb_d = nc.dram_tensor("fb", [128, 4, HW], BF16, kind="ExternalInput").ap()
    f8_d = nc.dram_tensor("f8", [128, 4, HW], F8, kind="ExternalInput").ap()
    wqk1_d = nc.dram_tensor("wqk1", [128, 2, 512], F8,
                            kind="ExternalInput").ap()
    wqk2_d = nc.dram_tensor("wqk2", [128, 2, 512], F8,
                            kind="ExternalInput").ap()
    wv_d = nc.dram_tensor("wv", [128, 2, 2, 256], F8, kind="ExternalInput").ap()
    gw_d = nc.dram_tensor("gw", [128, 4, 24, 128], BF16,
                          kind="ExternalInput").ap()
    expb_d = nc.dram_tensor("expb", [128, 256], BF16, kind="ExternalInput").ap()
    sev_d = nc.dram_tensor("sev", [128, 2], F32, kind="ExternalInput").ap()
    bve_d = nc.dram_tensor("bve", [128, 2, 2], F32, kind="ExternalInput").ap()
    bns_d = nc.dram_tensor("bns", [128, 2], F32, kind="ExternalInput").ap()
    bnb_d = nc.dram_tensor("bnb", [128, 2], F32, kind="ExternalInput").ap()
    muv_d = nc.dram_tensor("muv", [128, 1], F32, kind="ExternalInput").ap()
    y_d = nc.dram_tensor("y", [C, H, W], F32, kind="ExternalOutput").ap()

    with tile.TileContext(nc) as tc:
        with tc.tile_pool(name="per", bufs=1) as per, \
             tc.tile_pool(name="sml", bufs=4) as sml:

            # ---- persistent SBUF tiles ----
            wqk1 = per.tile([128, 2, 512], F8)
            wqk2 = per.tile([128, 2, 512], F8)
            wv = per.tile([128, 2, 2, 256], F8)
            gw = per.tile([128, 4, 24, 128], BF16)
            sev = per.tile([128, 2], F32)
            bve = per.tile([128, 2, 2], F32)
            bns = per.tile([128, 2], F32)
            bnb = per.tile([128, 2], F32)
            muv = per.tile([128, 1], F32)

            # DR ldweights needs a 16B-aligned k-subtile stride
            ones2t = per.tile([128, 2, 16], F8)
            nc.gpsimd.memset(ones2t, 1.0)
            ones2 = ones2t[:, :, 0:1]

            fp = [per.tile([128, PH, PW], BF16, tag=f"fp{j}", name=f"fp{j}")
                  for j in range(4)]

            vp_ctx = tc.tile_pool(name="vp", bufs=1)
            vp = vp_ctx.__enter__()
            f8 = vp.tile([128, 4, HW], F8)            # matmul operand F
            fbf = vp.tile([128, 4, HW], BF16)         # epilogue-add F (bf16)
            # qk8[:, n, 0:256]=16*Qs, [256:512]=exp(K1), [512:768]=exp(K2)
            qk8 = vp.tile([128, NT, 768], F8)
            vsb8 = vp.tile([128, 2, 2, HW], F8)       # fp8 V, (br, g, hw)
            att8 = vp.tile([128, 4, 2, 128], F8)      # DR lhsT per (2br+g)
            expb = vp.tile([128, 256], BF16)          # exp(bq), replicated

            # ---- preamble DMAs: f8 head + wqk are the critical path
            nc.sync.dma_start(f8[:, :, 0:256], f8_d[:, :, 0:256])
            nc.scalar.dma_start(wqk1, wqk1_d)
            nc.scalar.dma_start(wqk2, wqk2_d)
            nc.sync.dma_start(f8[:, :, 256:1024], f8_d[:, :, 256:1024])
            nc.scalar.dma_start(f8[:, :, 1024:2048], f8_d[:, :, 1024:2048])
            nc.gpsimd.dma_start(wv, wv_d)
            nc.gpsimd.dma_start(expb, expb_d)
            nc.scalar.dma_start(f8[:, :, 2048:3072], f8_d[:, :, 2048:3072])
            nc.sync.dma_start(f8[:, :, 3072:4096], f8_d[:, :, 3072:4096])
            nc.gpsimd.dma_start(sev, sev_d)
            nc.gpsimd.dma_start(bve, bve_d)
            nc.gpsimd.dma_start(muv, muv_d)
            # fbf not needed until the Xw epilogue; sync queue, after f8
            for ch in range(4):
                nc.sync.dma_start(fbf[:, :, ch * 1024:(ch + 1) * 1024],
                                  fb_d[:, :, ch * 1024:(ch + 1) * 1024])
            # fp pad-only memsets (rows 0,65 and cols 0,65)
            for j in range(4):
                nc.vector.memset(fp[j][:, 0:PH:PH - 1, 0:66], 0.0)
                nc.vector.memset(fp[j][:, 1:65, 0:66:65], 0.0)
            nc.vector.memset(att8, 0.0)

            # ============ Phase 1: QK exps + ssum + Att, all pipelined
            with tc.tile_pool(name="pq", bufs=2, space="PSUM") as pq, \
                 tc.tile_pool(name="pss", bufs=1, space="PSUM") as pss, \
                 tc.tile_pool(name="pv", bufs=2, space="PSUM") as pv, \
                 tc.tile_pool(name="pa", bufs=1, space="PSUM") as pa:

                ps_s = pss.tile([1, 512], F32, tag="s")
                psa = pa.tile([128, 512], F32, tag="a")

                LAG = 2

                def emit_ssum(p):
                    nc.tensor.matmul(ps_s, ones2,
                                     qk8[:, 2 * p:2 * p + 2, 256:768],
                                     start=(p == 0), stop=(p == NT // 2 - 1),
                                     perf_mode=DR)

                def emit_att(p):
                    for g in range(2):
                        nc.tensor.matmul(
                            psa[:, g * 256:(g + 1) * 256],
                            qk8[:, 2 * p:2 * p + 2, g * 128:(g + 1) * 128],
                            qk8[:, 2 * p:2 * p + 2,
                                256 + g * 256:256 + (g + 1) * 256],
                            start=(p == 0), stop=(p == NT // 2 - 1),
                            perf_mode=DR, skip_group_check=True)

                gk = dict(skip_group_check=True)
                expb3 = expb.rearrange("p (h e) -> p h e", h=NH)
                for n in range(NT):
                    # PSUM layout [Q | K1 | K2]: one 512-col [Q|K1] DR
                    # matmul fills bank A (pair0 lhsT); the pair1 lhsT then
                    # feeds a 256-col Q-accumulate and the K2 matmul (bank B,
                    # its own start). Matmul outs never cross a PSUM bank.
                    pqk = pq.tile([128, 768], F32, tag="qk")
                    psq = pqk[:, 0:256]
                    nsl = slice(n * 128, (n + 1) * 128)
                    nc.tensor.matmul(pqk[:, 0:512], f8[:, 0:2, nsl], wqk1,
                                     start=True, stop=False, perf_mode=DR, **gk)
                    nc.tensor.matmul(psq, f8[:, 2:4, nsl], wqk2[:, :, 0:256],
                                     start=False, stop=True, perf_mode=DR, **gk)
                    nc.tensor.matmul(pqk[:, 512:768], f8[:, 2:4, nsl],
                                     wqk2[:, :, 256:512],
                                     start=True, stop=False, perf_mode=DR, **gk)

                    # exp(K)/64 straight to fp8 (max ~122 < 240), writing the
                    # (g-major, br-minor) interleaved qk8 column layout
                    kin = pqk[:, 256:768].rearrange(
                        "p (b g c) -> p b g c", b=2, g=2)
                    kout = qk8[:, n, 256:768].rearrange(
                        "p (g b c) -> p b g c", g=2, b=2)
                    nc.scalar.activation(kout, kin, AF.Exp, scale=1.0 / WSC)
                    # exp(Q) to bf16; S_q is the exp(bq)-weighted sum
                    qbf = sml.tile([128, 256], BF16, tag="qbf")
                    actq = nc.scalar.activation(qbf, psq, AF.Exp,
                                                scale=1.0 / WSC)
                    if n == 8:
                        gw_anchor = actq
                    q3 = qbf.rearrange("p (h e) -> p h e", h=NH)
                    qw = sml.tile([128, 256], BF16, tag="qw")
                    qw3 = qw.rearrange("p (h e) -> p h e", h=NH)
                    nc.gpsimd.tensor_tensor(qw3, q3, expb3, ALU.mult)
                    rq = sml.tile([128, NH], F32, tag="rq")
                    nc.vector.tensor_reduce(rq, qw3, axis=AX.X, op=ALU.add)
                    rr = sml.tile([128, NH], F32, tag="rr")
                    nc.vector.reciprocal(rr, rq)
                    q83 = qk8[:, n, 0:256].rearrange("p (h e) -> p h e", h=NH)
                    nc.vector.scalar_tensor_tensor(
                        out=q83, in0=q3, scalar=QS_SCALE,
                        in1=rr.to_broadcast([128, NH, D]),
                        op0=ALU.mult, op1=ALU.mult)

                    if n >= 2 * LAG + 1 and n % 2 == 1:
                        p = (n - 1) // 2 - LAG
                        emit_ssum(p)
                        emit_att(p)
                for p in range(NT // 2 - LAG, NT // 2):
                    emit_ssum(p)
                    emit_att(p)

                # 1/S row -> per-partition columns (tiny SBUF->SBUF DMAs)
                scale = {}
                rs = sml.tile([1, 512], F32, tag="rs")
                nc.vector.reciprocal(rs, ps_s)
                for br in range(2):
                    for m in range(2):
                        col = sml.tile([128, 1], F32, tag="scat")
                        nc.sync.dma_start(
                            col, rs[0:1, m * 256 + br * 128:
                                    m * 256 + (br + 1) * 128])
                        sc = sml.tile([128, 1], F32, tag="scale")
                        nc.vector.tensor_mul(sc, col, muv)   # mu/(8*S_d)
                        scale[(br, m)] = sc

                # ======== Phase 2: V (fp8 DR) + att8 block-diag extraction
                def emit_v(br):
                    for g in range(2):
                        for n8 in range(8):
                            psv = pv.tile([128, 512], F32, tag="v",
                                          name=f"psv{br}{g}{n8}")
                            nc.tensor.matmul(
                                psv, wv[:, br, :, g * 128:(g + 1) * 128],
                                f8[:, 2 * br:2 * br + 2,
                                   n8 * 512:(n8 + 1) * 512],
                                start=True, stop=True, perf_mode=DR)
                            sl = slice(n8 * 512, (n8 + 1) * 512)
                            if n8 % 2 == 0:
                                nc.scalar.activation(
                                    vsb8[:, br, g, sl], psv, AF.Identity,
                                    bias=bve[:, br, g:g + 1],
                                    scale=sev[:, g:g + 1])
                            else:
                                nc.vector.scalar_tensor_tensor(
                                    out=vsb8[:, br, g, sl], in0=psv,
                                    scalar=sev[:, g:g + 1],
                                    in1=bve[:, br, g:g + 1].to_broadcast(
                                        [128, 512]),
                                    op0=ALU.mult, op1=ALU.add)

                emit_v(0)
                emit_v(1)

                for g in range(2):
                    for br in range(2):
                        for hb in range(4):
                            hs = slice(hb * 32, (hb + 1) * 32)
                            c0 = g * 256 + br * 128 + hb * 32
                            nc.vector.tensor_scalar_mul(
                                att8[hs, 2 * br + g, g, hs],
                                psa[hs, c0:c0 + 32], 1.0 / ATT_DS)

            # Winograd weights: start loading mid-phase-1 (nosync dep keeps
            # them out of the preamble DMA window), done before the conv.
            from concourse.tile import add_dep_helper
            for h in range(2):
                d = nc.sync.dma_start(gw[:, :, h * 12:(h + 1) * 12, :],
                                      gw_d[:, :, h * 12:(h + 1) * 12, :])
                add_dep_helper(d.ins, gw_anchor.ins, sync=False,
                               reason="defer gw load past preamble")
            d = nc.sync.dma_start(bns, bns_d)
            add_dep_helper(d.ins, gw_anchor.ins, sync=False, reason="defer")
            d = nc.sync.dma_start(bnb, bnb_d)
            add_dep_helper(d.ins, gw_anchor.ins, sync=False, reason="defer")

            # ============ Phase 2b: Xw (fp8 DR) + fused epilogue
            with tc.tile_pool(name="px", bufs=6, space="PSUM") as px:
                for br in range(2):
                    for g in range(2):
                        for n8 in range(8):
                            pxt = px.tile([128, 512], F32, tag="x")
                            nc.tensor.matmul(
                                pxt, att8[:, 2 * br + g, :, :],
                                vsb8[:, br, :, n8 * 512:(n8 + 1) * 512],
                                start=True, stop=True, perf_mode=DR)
                            # fused = (pxt * mu/(8 S_d)) + F -> padded img
                            j = 2 * br + g
                            out = fp[j][:, 1 + n8 * 8:9 + n8 * 8, 1:65]
                            nc.vector.scalar_tensor_tensor(
                                out=out,
                                in0=pxt, scalar=scale[(br, g)],
                                in1=fbf[:, j, n8 * 512:(n8 + 1) * 512],
                                op0=ALU.mult, op1=ALU.add)
            vp_ctx.__exit__(None, None, None)

            # ============ Phase 3: 1-D Winograd conv + BN + ReLU
            with tc.tile_pool(name="rp", bufs=1) as rp, \
                 tc.tile_pool(name="pc", bufs=8, space="PSUM") as pc, \
                 tc.tile_pool(name="ep", bufs=2) as ep:
                R = rp.tile([128, 4, 4, 32, RW], BF16)
                # R_k row combos: k0=d0-d2, k1=d1+d2, k2=d2-d1, k3=d1-d3.
                # Chunked per tyc so conv groups can start as R data lands,
                # and ci-major so early branches overlap the Xw phase.
                combos = [(0, 2, ALU.subtract), (1, 2, ALU.add),
                          (2, 1, ALU.subtract), (1, 3, ALU.subtract)]
                for ci in range(4):
                    for tyc in range(4):
                        for k, (a, b, op) in enumerate(combos):
                            r0 = a + 16 * tyc
                            r1 = b + 16 * tyc
                            nc.vector.tensor_tensor(
                                R[:, ci, k, tyc * 8:(tyc + 1) * 8, 0:66],
                                fp[ci][:, r0:min(r0 + 16, PH):2, 0:66],
                                fp[ci][:, r1:min(r1 + 16, PH):2, 0:66], op)

                groups = [(ty0, m, nty)
                          for ty0, nty in ((0, 8), (8, 8), (16, 8), (24, 8))
                          for m in range(2)]
                for ty0, m, nty in groups:
                    fw = nty * 64
                    pst_t = [pc.tile([128, 512], F32, tag="c",
                                     name=f"psc{m}{ty0}{k}")
                             for k in range(4)]
                    pst = [t[:, 0:fw] for t in pst_t]
                    for k in range(4):
                        for ci in range(4):
                            for dx in range(3):
                                nc.tensor.matmul(
                                    pst[k],
                                    gw[:, ci, k * 6 + dx * 2 + m, :],
                                    R[:, ci, k, ty0:ty0 + nty, dx:dx + 64],
                                    start=(ci == 0 and dx == 0),
                                    stop=(ci == 3 and dx == 2))
                    # A^T combine: y0 = M0+M1+M2, y1 = M1-M2-M3.
                    # DVE has one PSUM read port: stage M1 in SBUF so
                    # every TT reads at most one PSUM operand.
                    m1s_t = ep.tile([128, 512], F32, tag="m1s",
                                    name=f"m1s{m}{ty0}")
                    m1s = m1s_t[:, 0:fw]
                    nc.scalar.activation(m1s, pst[1], AF.Identity)
                    for i in range(2):
                        t_t = ep.tile([128, 512], F32, tag=f"t{i}",
                                      name=f"t{i}_{m}{ty0}")
                        t = t_t[:, 0:fw]
                        if i == 0:
                            nc.vector.tensor_tensor(t, pst[0], m1s, ALU.add)
                            s_t = ep.tile([128, 512], F32, tag="s0",
                                          name=f"s0_{m}{ty0}")
                            s = s_t[:, 0:fw]
                            nc.vector.tensor_tensor(s, t, pst[2], ALU.add)
                        else:
                            nc.vector.tensor_tensor(t, m1s, pst[2],
                                                    ALU.subtract)
                            s_t = ep.tile([128, 512], F32, tag="s1",
                                          name=f"s1_{m}{ty0}")
                            s = s_t[:, 0:fw]
                            nc.vector.tensor_tensor(s, t, pst[3],
                                                    ALU.subtract)
                        ysb_t = sml.tile([128, 512], F32, tag="y",
                                         name=f"y{m}{ty0}{i}")
                        ysb = ysb_t[:, 0:fw]
                        nc.scalar.activation(ysb, s, AF.Relu,
                                             bias=bnb[:, m:m + 1],
                                             scale=bns[:, m:m + 1])
                        eng = nc.sync if (ty0 + i) % 2 == 0 else nc.gpsimd
                        eng.dma_start(
                            y_d[m * 128:(m + 1) * 128,
                                ty0 * 2 + i:(ty0 + nty) * 2:2, :],
                            ysb.rearrange("p (a b) -> p a b", a=nty))
    nc.compile()
    return nc


_PROGRAM = None


def _get_program():
    global _PROGRAM
    if _PROGRAM is None:
        _PROGRAM = _build_program()
    return _PROGRAM


def kernel(F1, F2, Wq, bq, Wk1, bk1, Wv1, bv1, Wk2, bk2, Wv2, bv2,
           mu, Wc, gamma, beta, rmean, rvar):
    import os
    import sys
    if "antenv.axon_hooks" not in sys.modules:
        try:
            import antenv.axon_hooks  # noqa: F401
        except ImportError:
            # no profiling hook available: make sure a stray BASS_TRACE
            # can't route run_bass_kernel_spmd into the hook import
            os.environ["BASS_NEVER_TRACE"] = "1"
    f32 = np.float32
    F1 = np.asarray(F1, f32)
    F2 = np.asarray(F2, f32)

    def tile_T(w):   # [O, Cin] -> [128, Cin//128, O] f32 (lhsT tiles)
        wt = np.ascontiguousarray(np.asarray(w, f32).T)      # [Cin, O]
        cin, o = wt.shape
        return wt.reshape(cin // 128, 128, o).transpose(1, 0, 2)

    def q8(w):
        return np.ascontiguousarray(w * WSC).astype(_e4)

    wq_t = tile_T(Wq)                                        # [128, 4, 256]
    # fused phase-1 weights: rhs for [Q|K1] (F1-pair) and [Q|K2] (F2-pair)
    wqk1_h = q8(np.concatenate([wq_t[:, 0:2, :], tile_T(Wk1)], axis=2))
    wqk2_h = q8(np.concatenate([wq_t[:, 2:4, :], tile_T(Wk2)], axis=2))
    wv_h = q8(np.stack([tile_T(Wv1), tile_T(Wv2)], axis=1))

    Wc = np.asarray(Wc, f32)                                 # [256, 512, 3, 3]
    # 1-D Winograd dy-combos: G rows applied to the 3 dy taps
    g0 = Wc[:, :, 0, :]
    g1 = (Wc[:, :, 0, :] + Wc[:, :, 1, :] + Wc[:, :, 2, :]) * 0.5
    g2 = (Wc[:, :, 0, :] - Wc[:, :, 1, :] + Wc[:, :, 2, :]) * 0.5
    g3 = Wc[:, :, 2, :]
    G4 = np.stack([g0, g1, g2, g3])                          # [4k, 256, 512, 3]
    # gw[p, ci, k*6+dx*2+m, col] = G4[k, m*128+col, ci*128+p, dx]
    gw_h = G4.reshape(4, 2, 128, 4, 128, 3)                  # k,m,col,ci,p,dx
    gw_h = gw_h.transpose(4, 3, 0, 5, 1, 2)                  # p,ci,k,dx,m,col
    gw_h = np.ascontiguousarray(gw_h.reshape(128, 4, 24, 128)).astype(_bf)

    # exp(bq) factor: replicated row for the weighted S_q reduce, and
    # folded into the V copy scale/bias (Xw sums over the same e index)
    eb = np.exp(np.asarray(bq, f32))                         # [256]
    expb_h = np.ascontiguousarray(
        np.broadcast_to(eb[None, :], (128, 256))).astype(_bf)
    sev_h = np.ascontiguousarray(
        eb.reshape(2, 128).T * (V_US / WSC)).astype(f32)     # [128, 2]
    # bve[p, br, g] = V_US * eb[g*128+p] * bv_br[g*128+p]
    bvs = np.stack([np.asarray(bv1, f32), np.asarray(bv2, f32)], axis=0)
    bve_h = np.ascontiguousarray(
        (V_US * eb[None, :] * bvs).reshape(2, 2, 128).transpose(2, 0, 1))
    inv = np.asarray(gamma, f32) / np.sqrt(np.asarray(rvar, f32) + BN_EPS)
    b2 = np.asarray(beta, f32) - np.asarray(rmean, f32) * inv
    bns_h = np.ascontiguousarray(inv.reshape(2, 128).T)      # [128, 2]
    bnb_h = np.ascontiguousarray(b2.reshape(2, 128).T)
    # pxt = (QS_SCALE * V_US / ATT_DS) * S_K * Xw_true = 8 * S_K * Xw_true
    muv_h = np.full(
        (128, 1),
        np.asarray(mu, f32).reshape(-1)[0] * ATT_DS / (QS_SCALE * V_US),
        f32)

    shared = dict(wqk1=wqk1_h, wqk2=wqk2_h, wv=wv_h, gw=gw_h, expb=expb_h,
                  sev=sev_h, bve=bve_h, bns=bns_h, bnb=bnb_h, muv=muv_h)

    def packF(b):
        f1r = F1[b].reshape(C, HW)
        f2r = F2[b].reshape(C, HW)
        st = np.stack([f1r[:128], f1r[128:], f2r[:128], f2r[128:]], axis=1)
        return np.ascontiguousarray(st)                      # [128, 4, HW]

    in_maps = []
    for b in range(N_CORES):
        fb = packF(b)
        in_maps.append(dict(fb=fb.astype(_bf), f8=fb.astype(_e4), **shared))

    nc = _get_program()
    res = run_bass_kernel_spmd(nc, in_maps, list(range(N_CORES)))
    kernel.last_results = res

    out = np.stack([res.results[b]["y"] for b in range(N_CORES)])
    return out.reshape(B, C, H, W)


kernel.last_results = None
